# revision 11
# baseline (speedup 1.0000x reference)
"""Multi-head attention (B=2, S=2048, D=1024, H=16) on 8 trn2 NeuronCores.

Sharding: batch x head-group tensor parallel. Core c handles batch b=c//4 and
head group g=c%4 (4 heads = 256 features). Wq/Wk/Wv are split column-wise by
head (rows of the torch-layout weight), Wo row-wise; each core produces a
partial output for its batch which the host sums (row-parallel linear) and
adds bo.

Per-core dataflow (all matmuls f32r on the PE, f32 accumulation in PSUM):
  - host pre-transposes activations (x^T, d-major) and weight shards
  - Q^T,K^T = W^T.T @ x^T        [e on partitions]  (e-contraction for scores)
  - V       = x^T.T @ Wv^T       [s on partitions]  (+bias, +ones column)
  - S^T     = K^T_h.T @ Q^T_h    [k on partitions, K=64 contraction]
  - expS    = exp(0.125*S^T)     (ScalarE, straight from PSUM)
  - ctxU^T/den = V_aug.T @ expS  (ones column of V_aug produces den row)
  - ctx^T   = ctxU^T * bcast(1/den)   (PE ones-outer-product broadcast)
  - y      += ctx^T.T @ Wo^T     (accumulate 4 heads in PSUM)
"""

from contextlib import ExitStack

import numpy as np

import concourse.bass as bass
import concourse.tile as tile
from concourse import bacc, mybir

B, S, D, NH = 2, 2048, 1024, 16
NCORES = 8
GH = 4            # heads per core
DK = D // NH      # 64
E = GH * DK       # 256 local features per core
F32 = mybir.dt.float32
MM_DT = mybir.dt.float32r   # PE matmul dtype (f32r: full-rate reduced-precision)

QC = 512          # q-chunk (free dim of score tiles)
NQC = S // QC     # 4
NKB = S // 128    # 16 key blocks
NKD = D // 128    # 8 contraction panels for projections


def build_bass():
    nc = bacc.Bacc("TRN2", target_bir_lowering=False, debug=False,
                   num_devices=NCORES)

    xqT = nc.declare_dram_parameter("xqT", [D, S], MM_DT, isOutput=False)
    xkT = nc.declare_dram_parameter("xkT", [D, S], MM_DT, isOutput=False)
    xvT = nc.declare_dram_parameter("xvT", [D, S], MM_DT, isOutput=False)
    wqT = nc.declare_dram_parameter("wqT", [D, E], MM_DT, isOutput=False)
    wkT = nc.declare_dram_parameter("wkT", [D, E], MM_DT, isOutput=False)
    wvT = nc.declare_dram_parameter("wvT", [D, E], MM_DT, isOutput=False)
    bq2 = nc.declare_dram_parameter("bq2", [128, 2], F32, isOutput=False)
    bk2 = nc.declare_dram_parameter("bk2", [128, 2], F32, isOutput=False)
    bvb = nc.declare_dram_parameter("bvb", [128, E], F32, isOutput=False)
    woT = nc.declare_dram_parameter("woT", [DK, GH, D], MM_DT, isOutput=False)
    ones1 = nc.declare_dram_parameter("ones1", [128, DK], MM_DT,
                                      isOutput=False)
    vones = nc.declare_dram_parameter("vones", [128, NKB * GH], MM_DT,
                                      isOutput=False)
    y = nc.declare_dram_parameter("y", [S, D], F32, isOutput=True)

    with ExitStack() as ctx:
        tc = ctx.enter_context(tile.TileContext(nc))
        const = ctx.enter_context(tc.tile_pool(name="const", bufs=1))
        persist = ctx.enter_context(tc.tile_pool(name="persist", bufs=1))
        xt = ctx.enter_context(tc.tile_pool(name="xt", bufs=10))
        exps_p = ctx.enter_context(tc.tile_pool(name="exps", bufs=3))
        small = ctx.enter_context(tc.tile_pool(name="small", bufs=2))
        outp = ctx.enter_context(tc.tile_pool(name="outp", bufs=2))
        ps_proj = ctx.enter_context(
            tc.tile_pool(name="ps_proj", bufs=2, space="PSUM"))
        ps_s = ctx.enter_context(
            tc.tile_pool(name="ps_s", bufs=2, space="PSUM"))
        ps_c = ctx.enter_context(
            tc.tile_pool(name="ps_c", bufs=2, space="PSUM"))

        # ---- constants / weights ----
        wq_sb = const.tile([128, NKD, E], MM_DT, tag="wq")
        wk_sb = const.tile([128, NKD, E], MM_DT, tag="wk")
        wv_sb = const.tile([128, NKD, E], MM_DT, tag="wv")
        nc.sync.dma_start(wq_sb[:], wqT[:].rearrange("(k p) e -> p k e", p=128))
        nc.sync.dma_start(wk_sb[:], wkT[:].rearrange("(k p) e -> p k e", p=128))
        nc.sync.dma_start(wv_sb[:], wvT[:].rearrange("(k p) e -> p k e", p=128))
        wo_sb = const.tile([DK, GH, D], MM_DT, tag="wo")
        nc.sync.dma_start(wo_sb[:], woT[:])
        bias_q = const.tile([128, 2], F32, tag="bq")
        bias_k = const.tile([128, 2], F32, tag="bk")
        bv_bc = const.tile([128, E], F32, tag="bv")
        nc.sync.dma_start(bias_q[:], bq2[:])
        nc.sync.dma_start(bias_k[:], bk2[:])
        nc.sync.dma_start(bv_bc[:], bvb[:])
        ones_col = const.tile([128, DK], MM_DT, tag="ones")
        nc.sync.dma_start(ones_col[:], ones1[:])

        QT_sb = persist.tile([128, 2, S], MM_DT, tag="qt")
        KT_sb = persist.tile([128, 2, S], MM_DT, tag="kt")
        V_aug = persist.tile([128, NKB, GH, DK + 1], MM_DT, tag="va")
        nc.sync.dma_start(V_aug[:, :, :, DK:DK + 1], vones[:])
        ctxT = persist.tile([DK, GH, S], MM_DT, tag="ctx")

        # ---- Q^T / K^T projections (x^T streamed in S-halves) ----
        SH = S // 2
        for src, wsb, bias, dst in ((xqT, wq_sb, bias_q, QT_sb),
                                    (xkT, wk_sb, bias_k, KT_sb)):
            for half in range(2):
                panels = []
                for kd in range(NKD):
                    p = xt.tile([128, SH], MM_DT, tag="xt")
                    nc.sync.dma_start(
                        p[:], src[kd * 128:(kd + 1) * 128,
                                  half * SH:(half + 1) * SH])
                    panels.append(p)
                for t in range(2):
                    for qc in range(half * 2, half * 2 + 2):
                        ps = ps_proj.tile([128, QC], F32, tag="psp")
                        lo = qc * QC - half * SH
                        for kd in range(NKD):
                            nc.tensor.matmul(
                                ps[:],
                                wsb[:, kd, t * 128:(t + 1) * 128],
                                panels[kd][:, lo:lo + QC],
                                start=(kd == 0), stop=(kd == NKD - 1))
                        nc.vector.tensor_scalar_add(
                            dst[:, t, qc * QC:(qc + 1) * QC], ps[:],
                            bias[:, t:t + 1])

        # ---- V projection (natural layout, +bias, +ones col) ----
        for half in range(2):
            panels_v = []
            for kd in range(NKD):
                p = xt.tile([128, SH], MM_DT, tag="xt")
                nc.sync.dma_start(
                    p[:], xvT[kd * 128:(kd + 1) * 128,
                              half * SH:(half + 1) * SH])
                panels_v.append(p)
            for st in range(half * 8, half * 8 + 8):
                lo = st * 128 - half * SH
                ps = ps_proj.tile([128, QC], F32, tag="psp")
                for kd in range(NKD):
                    nc.tensor.matmul(
                        ps[:, 0:E],
                        panels_v[kd][:, lo:lo + 128],
                        wv_sb[:, kd, :],
                        start=(kd == 0), stop=(kd == NKD - 1))
                nc.vector.tensor_tensor(
                    out=V_aug[:, st, :, 0:DK], in0=ps[:, 0:E], in1=bv_bc[:],
                    op=mybir.AluOpType.add)

        # ---- attention + normalize ----
        # Head pairs (2t, 2t+1) interleave: their K=64 score matmuls sit at
        # partition bases 0/64 -> different PE row groups -> concurrent.
        # Score kb-pairs share one [128, 2*QC] PSUM tile so exp runs at
        # FD=1024 (halves ScalarE per-call overhead).
        for qc in range(NQC):
            qsl = slice(qc * QC, (qc + 1) * QC)
            for t in range(2):
                psc_a = ps_c.tile([DK + 1, QC], F32, tag="psc")
                psc_b = ps_c.tile([DK + 1, QC], F32, tag="psc")
                pscs = [psc_a, psc_b]
                for kp in range(NKB // 2):
                    esb = []
                    for hp in range(2):
                        esl = slice(hp * 64, (hp + 1) * 64)
                        pss = ps_s.tile([128, 2, QC], F32, tag="pss")
                        for j in range(2):
                            kb = 2 * kp + j
                            nc.tensor.matmul(
                                pss[:, j, :],
                                KT_sb[esl, t, kb * 128:(kb + 1) * 128],
                                QT_sb[esl, t, qsl])
                        es = exps_p.tile([128, 2, QC], MM_DT, tag="es")
                        nc.scalar.activation(
                            es[:], pss[:], mybir.ActivationFunctionType.Exp,
                            scale=float(1.0 / np.sqrt(DK)))
                        esb.append(es)
                    for hp in range(2):
                        h = 2 * t + hp
                        for j in range(2):
                            kb = 2 * kp + j
                            nc.tensor.matmul(
                                pscs[hp][:], V_aug[:, kb, h, :],
                                esb[hp][:, j, :],
                                start=(kp == 0 and j == 0),
                                stop=(kp == NKB // 2 - 1 and j == 1))
                for hp in range(2):
                    h = 2 * t + hp
                    psc = pscs[hp]
                    # 1/den as exp(-ln(den)), kept at partition 64 throughout
                    # (engines cannot move data across partitions).
                    lnd = small.tile([128, QC], F32, tag="lnd")
                    nc.scalar.activation(lnd[DK:DK + 1, :], psc[DK:DK + 1, :],
                                         mybir.ActivationFunctionType.Ln)
                    rdr = small.tile([128, QC], MM_DT, tag="rdr")
                    nc.scalar.activation(rdr[DK:DK + 1, :], lnd[DK:DK + 1, :],
                                         mybir.ActivationFunctionType.Exp,
                                         scale=-1.0)
                    psb = ps_proj.tile([128, QC], F32, tag="psp")
                    nc.tensor.matmul(psb[0:DK, :], ones_col[DK:DK + 1, :],
                                     rdr[DK:DK + 1, :])
                    cu = small.tile([DK, QC], F32, tag="cu")
                    nc.vector.tensor_copy(cu[:], psc[0:DK, :])
                    nc.vector.tensor_tensor(
                        out=ctxT[:, h, qsl], in0=cu[:], in1=psb[0:DK, :],
                        op=mybir.AluOpType.mult)

            # ---- output projection for this q-chunk ----
            for sti in range(QC // 128):
                st = qc * (QC // 128) + sti
                ssl = slice(st * 128, (st + 1) * 128)
                ob = outp.tile([128, D], F32, tag="ob")
                for oc in range(2):
                    pso = ps_proj.tile([128, QC], F32, tag="psp")
                    for h in range(GH):
                        nc.tensor.matmul(
                            pso[:],
                            ctxT[:, h, ssl],
                            wo_sb[:, h, oc * 512:(oc + 1) * 512],
                            start=(h == 0), stop=(h == GH - 1))
                    nc.vector.tensor_copy(ob[:, oc * 512:(oc + 1) * 512], pso[:])
                nc.sync.dma_start(y[ssl, :], ob[:])

    nc.compile()
    return nc


def make_in_maps(query, key, value, Wq, bq, Wk, bk, Wv, bv, Wo, bo):
    query = np.asarray(query, np.float32)
    key = np.asarray(key, np.float32)
    value = np.asarray(value, np.float32)
    Wq, Wk, Wv, Wo = (np.asarray(w, np.float32) for w in (Wq, Wk, Wv, Wo))
    bq, bk, bv = (np.asarray(b_, np.float32) for b_ in (bq, bk, bv))
    in_maps = []
    xT = {}
    for b in range(B):
        xT[b] = (np.ascontiguousarray(query[b].T),
                 np.ascontiguousarray(key[b].T),
                 np.ascontiguousarray(value[b].T))
    for c in range(NCORES):
        b, g = divmod(c, GH)
        sl = slice(g * E, (g + 1) * E)
        qT, kT, vT = xT[b]
        in_maps.append({
            "xqT": qT, "xkT": kT, "xvT": vT,
            "wqT": np.ascontiguousarray(Wq[sl, :].T),
            "wkT": np.ascontiguousarray(Wk[sl, :].T),
            "wvT": np.ascontiguousarray(Wv[sl, :].T),
            "bq2": np.ascontiguousarray(bq[sl].reshape(2, 128).T),
            "bk2": np.ascontiguousarray(bk[sl].reshape(2, 128).T),
            "bvb": np.ascontiguousarray(np.tile(bv[sl][None, :], (128, 1))),
            "woT": np.ascontiguousarray(
                Wo[:, sl].T.reshape(GH, DK, D).transpose(1, 0, 2)),
            "ones1": np.ones((128, DK), np.float32),
            "vones": np.ones((128, NKB * GH), np.float32),
        })
    return in_maps


_NC_CACHE = {}


def _get_nc():
    if "nc" not in _NC_CACHE:
        _NC_CACHE["nc"] = build_bass()
    return _NC_CACHE["nc"]


def kernel(query, key, value, Wq, bq, Wk, bk, Wv, bv, Wo, bo, **_):
    from concourse import bass_utils

    nc = _get_nc()
    in_maps = make_in_maps(query, key, value, Wq, bq, Wk, bk, Wv, bv, Wo, bo)
    res = bass_utils.run_bass_kernel_spmd(nc, in_maps, list(range(NCORES)))
    parts = [np.asarray(r["y"], np.float32) for r in res.results]
    bo = np.asarray(bo, np.float32)
    out = np.empty((B, S, D), np.float32)
    for b in range(B):
        out[b] = parts[4 * b] + parts[4 * b + 1] + parts[4 * b + 2] \
            + parts[4 * b + 3] + bo
    return out
